# revision 1
# baseline (speedup 1.0000x reference)
"""Attention-pooling kernel for TRN2 (8 NeuronCores, data-parallel over batch).

Problem (nn_AttentionPooling3): x [16, 4096, 1024] f32; per head h of 8,
logit[b,h,t] = x[b,t,h*128:(h+1)*128] @ (Q[h] @ key_p[h]) / sqrt(64);
attn = softmax over t; out[b, h*128:(h+1)*128] = sum_t attn * x-slice.

Strategy per core (2 batches/core):
- Q/key_p fold to a single weight row w [1024] on host (scale included),
  replicated to all 128 partitions and passed as input "wb".
- x streamed once in natural layout: units of 512 rows as [128p(T), 4, 1024]
  (contiguous 4KB DMA bursts).
- Logits: GPSIMD does the elementwise x*wb product, DVE reduces each
  128-wide head block ([128, 4, 8, 128] -> [128, 4, 8]) — one op each per
  unit, splitting the work across two otherwise-idle engines so DMA stays
  the bottleneck.
- exp on ScalarE (no max subtraction: softmax is shift-invariant and the
  logit range |l| < 60 keeps exp well within f32 range).
- Weighted sum + normalizer on the PE: lhsT = e [128,8] stationary,
  rhs = x chunk [128, 512]x2 and a ones column; accumulated in PSUM over
  all 32 chunks; final scale by 1/s on ScalarE.
Host extracts the per-head diagonal blocks of the [8, 1024] PE output.
"""

import math

import numpy as np

import concourse.bass as bass
import concourse.mybir as mybir
import concourse.tile as tile
from concourse.bass_utils import run_bass_kernel_spmd

B, T, F = 16, 4096, 1024
H, V, KD = 8, 128, 64
NCORES = 8
BL = B // NCORES            # batches per core: 2
NCH = 4                     # 128-row chunks per unit
UNIT = 128 * NCH            # 512 rows
NUNITS = T // UNIT          # 8
NCHUNKS = T // 128          # 32
FP32 = mybir.dt.float32


def _build_nc():
    nc = bass.Bass()
    x_d = nc.declare_dram_parameter("x", [BL, T, F], FP32, isOutput=False)
    wb_d = nc.declare_dram_parameter("wb", [128, F], FP32, isOutput=False)
    y_d = nc.declare_dram_parameter("y", [BL, H, F], FP32, isOutput=True)

    with tile.TileContext(nc) as tc:
        with (
            tc.tile_pool(name="const", bufs=1) as const_pool,
            tc.tile_pool(name="xin", bufs=4) as xpool,
            tc.tile_pool(name="prod", bufs=3) as ppool,
            tc.tile_pool(name="small", bufs=4) as small,
            tc.tile_pool(name="yout", bufs=2) as ypool,
            tc.tile_pool(name="acc", bufs=2, space="PSUM") as psum_pool,
        ):
            # wb loads once (512KB, Scalar HWDGE queue so it doesn't delay
            # unit 0's x load on the Sync queue); the multiply reads it
            # through a 0-step AP that repeats it NCH times along free.
            wb_sb = const_pool.tile([128, F], FP32)
            nc.scalar.dma_start(out=wb_sb, in_=wb_d[:, :])
            ones_sb = const_pool.tile([128, 1], FP32)
            nc.vector.memset(ones_sb, 1.0)

            # Work items per batch: (first-128-chunk, n-chunks, engine).
            # GPSIMD owns the steady-state multiplies, DVE the reduces: GP's
            # 2-read TT (0.94 rd/cyc) + DVE's 1-read reduce exactly saturate
            # the shared 2-port SBUF read complex. DVE TTs emitted anywhere
            # before the last GP TT flip the whole kernel into a ~20%-slower
            # arbitration mode (measured on 3 schedules), so DVE multiplies
            # come only AFTER the GP chain, at the very end of batch 1.
            # Boundary units are split in half: the first item needs only a
            # 1MB DMA before the GP chain starts; the last GP item halves
            # the final reduce's lag; the DVE tail items pipeline their
            # exp/matmuls at finer grain.
            def items_for(b):
                its = []
                if b == 0:
                    its += [(0, 1, "gp"), (1, 3, "gp")]
                    its += [(4 * u, 4, "gp") for u in range(1, NUNITS)]
                else:
                    its += [(4 * u, 4, "gp") for u in range(0, NUNITS - 3)]
                    its += [(20, 2, "gp"), (22, 2, "gp")]
                    its += [(c, 2, "ve") for c in (24, 26, 28, 30)]
                return its

            for b in range(BL):
                pooled_ps = psum_pool.tile([H, F], FP32)
                s_ps = psum_pool.tile([H, 1], FP32)
                items = items_for(b)
                for it_idx, (ch0, nch, eng) in enumerate(items):
                    xt = xpool.tile([128, NCH, F], FP32, name="xt")
                    xt_v = xt[:, :nch, :]
                    # All x loads on the Sync HWDGE queue: Scalar-queue
                    # dispatches would delay exp/normalize on ScalarE.
                    nc.sync.dma_start(
                        out=xt_v,
                        in_=x_d[
                            b, ch0 * 128 : (ch0 + nch) * 128, :
                        ].rearrange("(n p) f -> p n f", p=128),
                    )
                    prod = ppool.tile([128, NCH, F], FP32, name="prod")
                    prod_v = prod[:, :nch, :]
                    wb_bc = bass.AP(
                        tensor=wb_sb.tensor,
                        offset=wb_sb.offset,
                        ap=[wb_sb.ap[0], [0, nch], wb_sb.ap[1]],
                    )
                    mul_eng = nc.vector if eng == "ve" else nc.gpsimd
                    mul_eng.tensor_mul(prod_v, xt_v, wb_bc)
                    logits_u = small.tile([128, NCH, H], FP32, name="logits_u")
                    nc.vector.tensor_reduce(
                        logits_u[:, :nch, :],
                        prod_v.rearrange("p n (h v) -> p n h v", v=V),
                        axis=mybir.AxisListType.X,
                        op=mybir.AluOpType.add,
                    )
                    e_u = small.tile([128, NCH, H], FP32, name="e_u")
                    nc.scalar.activation(
                        out=e_u[:, :nch, :],
                        in_=logits_u[:, :nch, :],
                        func=mybir.ActivationFunctionType.Exp,
                    )
                    # Group matmuls by PSUM bank (all chunks' low halves,
                    # then all high halves): per-MM bank alternation causes
                    # HAM re-throttle and blocks MM pipelining.
                    for half in range(2):
                        lo, hi = half * 512, half * 512 + 512
                        for n in range(nch):
                            ch = ch0 + n
                            first, last = ch == 0, ch == NCHUNKS - 1
                            nc.tensor.matmul(
                                pooled_ps[:, lo:hi],
                                e_u[:, n, :],
                                xt[:, n, lo:hi],
                                start=first,
                                stop=last,
                            )
                    if eng == "ve":
                        # Tail items: per-chunk N=1 normalizer matmuls on the
                        # (idle) PE. An e_sum reduce here would sit on the
                        # serial DVE tail chain waiting for ScalarE's exp —
                        # ~2us of cross-engine latency per item.
                        for n in range(nch):
                            nc.tensor.matmul(
                                s_ps,
                                e_u[:, n, :],
                                ones_sb,
                                start=False,
                                stop=ch0 + n == NCHUNKS - 1,
                            )
                    else:
                        # Steady state: one head-sum reduce (DVE has slack
                        # here) feeds a single normalizer matmul per item.
                        e_sum_u = small.tile([128, H], FP32, name="e_sum_u")
                        nc.vector.tensor_reduce(
                            e_sum_u,
                            e_u[:, :nch, :].rearrange("p n h -> p h n"),
                            axis=mybir.AxisListType.X,
                            op=mybir.AluOpType.add,
                        )
                        gp_items = [i for i in items if i[2] == "gp"]
                        nc.tensor.matmul(
                            s_ps,
                            e_sum_u,
                            ones_sb,
                            start=it_idx == 0,
                            stop=(ch0, nch, eng) == gp_items[-1]
                            and items[-1][2] == "gp",
                        )
                r_sb = small.tile([H, 1], FP32)
                nc.vector.reciprocal(r_sb, s_ps)
                y_sb = ypool.tile([H, F], FP32)
                nc.scalar.activation(
                    out=y_sb,
                    in_=pooled_ps,
                    func=mybir.ActivationFunctionType.Copy,
                    scale=r_sb,
                )
                nc.sync.dma_start(out=y_d[b], in_=y_sb)
    return nc


def _split_multiwaits(nc, limit=1):
    """This container's walrus accepts at most `limit` sync-wait commands per
    instruction ("Too many sync wait commands" otherwise). Tile attaches up to
    ~12. Move excess waits onto preceding same-engine NoOps — semantics are
    unchanged (waits are AND conditions that block the engine either way)."""
    for fn in nc.m.functions:
        for blk in fn.blocks:
            new = []
            for inst in blk.instructions:
                si = getattr(inst, "sync_info", None)
                ow = list(si.on_wait) if si is not None and si.on_wait else []
                if len(ow) > limit:
                    extra, keep = ow[:-limit], ow[-limit:]
                    for i in range(0, len(extra), limit):
                        new.append(
                            mybir.InstNoOp(
                                name=f"{inst.name}-wsplit{i}",
                                engine=inst.engine,
                                ins=[],
                                outs=[],
                                sync_info=mybir.SyncInfo(
                                    on_wait=extra[i : i + limit], on_update=[]
                                ),
                            )
                        )
                    inst.sync_info = mybir.SyncInfo(
                        on_wait=keep, on_update=si.on_update
                    )
                new.append(inst)
            blk.instructions = new


_NC = None


def _get_nc():
    global _NC
    if _NC is None:
        _NC = _build_nc()
        _split_multiwaits(_NC)
    return _NC


def _fold_weights(Q, key_p):
    w = np.einsum(
        "hvk,hk->hv", np.asarray(Q, np.float32), np.asarray(key_p, np.float32)[:, :, 0]
    ) / np.float32(math.sqrt(KD))
    return np.tile(w.reshape(1, H * V).astype(np.float32), (128, 1))


def _run(x, Q, key_p, trace=False, tmpdir=None):
    x = np.ascontiguousarray(np.asarray(x, np.float32))
    wb = _fold_weights(Q, key_p)
    nc = _get_nc()
    in_maps = [
        {"x": x[c * BL : (c + 1) * BL], "wb": wb} for c in range(NCORES)
    ]
    res = run_bass_kernel_spmd(
        nc, in_maps, list(range(NCORES)), trace=trace, tmpdir=tmpdir
    )
    y = np.empty((B, F), np.float32)
    for c in range(NCORES):
        yc = res.results[c]["y"]  # [BL, H, F]
        for b in range(BL):
            for h in range(H):
                y[c * BL + b, h * V : (h + 1) * V] = yc[b, h, h * V : (h + 1) * V]
    return y, res


def kernel(**inputs):
    y, _ = _run(inputs["x"], inputs["Q"], inputs["key_p"])
    return y

